# revision 20
# baseline (speedup 1.0000x reference)
"""GAT (2-layer, PyG-style GATConv) on 8 Trainium2 NeuronCores.

Strategy (dst-sharded, quad-packed gather):
- Nodes sharded by dst across 8 cores (12500 each); edges partitioned by dst
  core; segment-softmax + weighted aggregation local per dst shard.
- Node table packs FOUR nodes per 256B row ([16 bf16 h | fp32 a_src | pad] x4),
  so a single int16-indexed SWDGE gather chunk covers all 100k nodes
  (idx = node>>2, 25000 rows). One gather per 128-dst tile fetches one 256B
  quad-row per edge slot; per-slot additive masks (0 / ln(mult) / -1e30)
  select the sub-row inside the segment softmax, which also handles duplicate
  edges exactly.
- Slots are degree-sorted per core; the static grid K[t] is the max over
  cores, while each core's gather stops early via a runtime num_idxs register
  (trailing -1 idxs are skipped by SWDGE), so descriptor count ~= edge count
  (+0.4%).
- 3 SPMD launches: transform (x@W1 + scores) / layer-1 aggregation /
  layer-2 aggregation + classifier + log_softmax. The gather schedule is
  identical for both layers (same position map), so idx/mask/count tensors
  are built once and reused.
"""

import numpy as np

import concourse.ap_utils as ap_utils
import concourse.bacc as bacc
import concourse.bass as bass
import concourse.mybir as mybir
from concourse.bass import round_up_to_multiple
from concourse.bass_utils import run_bass_kernel_spmd
from concourse.masks import make_identity
from concourse.tile import TileContext

P = 128
NCORES = 8
N = 100000
F_IN = 512
HID = 16
C_OUT = 32
NEG_SLOPE = 0.2
NROW = N // 4          # 25000 quad rows
ROWW = 128             # bf16 elems per table row (256B)
SUBW = 32              # bf16 elems per sub-row (64B)
SH = N // NCORES       # nodes per core
T_TILES = (SH + P - 1) // P
SHP = T_TILES * P      # padded shard size (12544)
NEG_BIG = -1.0e30
MAX_IDX_PER_GATHER = 8192

FP = mybir.dt.float32
BF = mybir.dt.bfloat16
I16 = mybir.dt.int16
I32 = mybir.dt.int32


def _my_dma_gather(gp, out_ap, in_ap, idxs_ap, num_idxs, num_idxs_reg,
                   elem_size, elem_step, queue_num):
    """BassGpSimd.dma_gather (non-transpose, DRAM source) without the
    256B-elem_size restriction and with a runtime num_idxs register; the row
    stride (elem_step) must still be a multiple of 256B."""
    assert idxs_ap.dtype == I16
    assert in_ap.dtype == out_ap.dtype
    assert in_ap.space == bass.MemorySpace.DRAM
    assert idxs_ap.space == bass.MemorySpace.SBUF
    assert out_ap.space == bass.MemorySpace.SBUF
    assert ap_utils.ap_is_contiguous(out_ap.ap[1:])
    assert ap_utils.ap_is_contiguous(idxs_ap.ap[1:])
    assert in_ap.ap[-1][1] == out_ap.ap[-1][1] == elem_size
    assert out_ap.ap[0][1] * out_ap.ap[1][1] == round_up_to_multiple(num_idxs, 128)
    assert in_ap.ap[0][0] == elem_step
    stride_bytes = elem_step * mybir.dt.size(in_ap.dtype)
    assert stride_bytes % 256 == 0 and stride_bytes // 256 < 256
    _in_ap = gp.lower_ap_dma(in_ap, for_custom_bir_dma=True)
    _idxs_ap = gp.lower_ap(idxs_ap)
    _out_ap = gp.lower_ap(out_ap)
    return gp.add_instruction(
        mybir.InstDMAGatherAnt(
            name=gp.bass.get_next_instruction_name(),
            ins=[*_in_ap, _idxs_ap, gp.lower_val_access(gp.to_reg(num_idxs_reg))],
            outs=[_out_ap],
            transpose=False,
            num_idxs=num_idxs,
            elem_size=elem_size,
            stride_bytes_256=stride_bytes // 256,
            gen_mode=0,
            single_packet=False,
            queue_num=queue_num,
        )
    )


# ---------------------------------------------------------------------------
# Host-side preprocessing
# ---------------------------------------------------------------------------

def _wrap_idx(local_idx):
    """Wrap an int16 index list [M] (M % 128 == 0) into the SWDGE layout
    [128, M//16]: idx i at partition i%16, col i//16, replicated x8."""
    M = local_idx.shape[0]
    w = local_idx.reshape(M // 16, 16).T.astype(np.int16)  # [16, M//16]
    return np.tile(w, (8, 1))


def _build_schedule(edge_index):
    """Shared (both layers) gather schedule.

    Returns dict with per-core: order (node order within shard), idx tensor
    [128, sum(K)*8] i16, mask tensor [128, sum(K)*4] f32, counts [1, T] i32,
    and shared K [T] plus slot offsets.
    """
    src = np.asarray(edge_index[0], dtype=np.int64)
    dst = np.asarray(edge_index[1], dtype=np.int64)
    loops = np.arange(N, dtype=np.int64)
    src = np.concatenate([src, loops])
    dst = np.concatenate([dst, loops])
    core = dst // SH

    per_core = []
    for c in range(NCORES):
        m = core == c
        s_c = src[m]
        d_loc = dst[m] - c * SH
        quad = s_c >> 2
        sub = (s_c & 3).astype(np.int64)
        # slots: unique (d_loc, quad); per-(slot, sub) multiplicity
        ekey = (d_loc * NROW + quad) * 4 + sub
        uk, ucnt = np.unique(ekey, return_counts=True)
        skey = uk >> 2
        usub = (uk & 3).astype(np.int64)
        slot_ids, slot_inv = np.unique(skey, return_inverse=True)
        nslots = len(slot_ids)
        slot_d = slot_ids // NROW
        slot_q = (slot_ids % NROW).astype(np.int64)
        # mask values [nslots, 4]
        mask = np.full((nslots, 4), NEG_BIG, np.float32)
        mask[slot_inv, usub] = np.log(ucnt).astype(np.float32)
        deg = np.bincount(slot_d, minlength=SH)
        per_core.append((slot_d, slot_q, mask, deg))

    # shared degree-sorted tiling
    orders = [np.argsort(-pc[3], kind="stable").astype(np.int64)
              for pc in per_core]
    Kc = np.zeros((NCORES, T_TILES), np.int64)
    for c in range(NCORES):
        deg = per_core[c][3]
        ds = deg[orders[c]]
        grid = np.concatenate([ds, np.zeros(SHP - SH, np.int64)]).reshape(T_TILES, P)
        Kc[c] = grid.max(axis=1)
    K = np.maximum(Kc.max(axis=0), 1)
    assert int(K.max()) * P <= MAX_IDX_PER_GATHER
    off = np.concatenate([[0], np.cumsum(K)])  # slot-col offsets per tile

    cores_data = []
    for c in range(NCORES):
        slot_d, slot_q, mask, deg = per_core[c]
        order = orders[c]
        gridpos = np.full(SH, -1, np.int64)
        gridpos[order] = np.arange(SH)
        gp_s = gridpos[slot_d]          # grid position of each slot's dst
        t_s = gp_s // P
        p_s = gp_s % P
        # rank of slot within its (dst) list
        so = np.argsort(gp_s * NROW + slot_q, kind="stable")
        gs = gp_s[so]
        rank = np.arange(len(gs)) - np.searchsorted(gs, gs, side="left")
        rank_s = np.empty_like(rank)
        rank_s[so] = rank

        total = int(K.sum()) * P
        idx_arr = np.full(total, -1, np.int64)
        mask_arr = np.full((total, 4), NEG_BIG, np.float32)
        pos = (off[t_s] + rank_s) * P + p_s
        idx_arr[pos] = slot_q
        mask_arr[pos] = mask

        # per-tile: convert interior -1 to 0 (gathered, masked), count
        counts = np.zeros(T_TILES, np.int32)
        for t in range(T_TILES):
            a = idx_arr[off[t] * P:off[t + 1] * P]
            nz = np.nonzero(a >= 0)[0]
            last = int(nz.max()) if len(nz) else 0
            a[:last + 1][a[:last + 1] < 0] = 0
            counts[t] = last + 1
        idx_t = _wrap_idx(idx_arr.astype(np.int16))           # [128, sumK*8]
        # mask tensor laid out [128, sumK*4]: partition p, tile t, slot k, sub
        mask_t = np.ascontiguousarray(
            mask_arr.reshape(int(K.sum()), P, 4).transpose(1, 0, 2).reshape(P, -1))
        cnt_t = np.zeros((1, T_TILES), np.int32)
        cnt_t[0] = counts
        cores_data.append({"idx": idx_t, "mask": mask_t, "counts": cnt_t,
                           "order": order})
    return K, cores_data


# ---------------------------------------------------------------------------
# Device programs
# ---------------------------------------------------------------------------

def _build_transform(repeat=1):
    """Launch 1: per core, h = xT_shard.T @ W1, a_s = h@att_src, a_d = h@att_dst.
    Inputs : xt [F_IN, SH] bf16 (pre-transposed shard), w1 [F_IN//P, P, HID] bf16,
             att [128, 2*HID] fp32 (att_src tiled | att_dst tiled)
    Outputs: hb [SHP, HID] bf16, asd [SHP, 2] fp32 (a_s | a_d)
    """
    nc = bacc.Bacc("TRN2", target_bir_lowering=False, debug=False,
                   num_devices=NCORES)
    xt = nc.dram_tensor("xt", [F_IN, SH], BF, kind="ExternalInput").ap()
    w1 = nc.dram_tensor("w1", [F_IN // P, P, HID], BF, kind="ExternalInput").ap()
    att = nc.dram_tensor("att", [P, 2 * HID], FP, kind="ExternalInput").ap()
    hb = nc.dram_tensor("hb", [SHP, HID], BF, kind="ExternalOutput").ap()
    asd = nc.dram_tensor("asd", [SHP, 2], FP, kind="ExternalOutput").ap()
    KC = F_IN // P
    with TileContext(nc) as tc:
        with tc.tile_pool(name="cst", bufs=1) as cst, \
             tc.tile_pool(name="xk", bufs=3) as xk, \
             tc.tile_pool(name="hp", bufs=3) as hp, \
             tc.tile_pool(name="ps", bufs=2, space="PSUM") as ps:
            w1t = cst.tile([P, KC * HID], BF)
            nc.sync.dma_start(out=w1t[:].rearrange("p (k h) -> p k h", k=KC),
                              in_=w1[:].rearrange("k p h -> p k h"))
            attt = cst.tile([P, 2 * HID], FP)
            nc.sync.dma_start(out=attt[:], in_=att[:])

            def tbody(t):
                m0 = t * P
                mn = min(P, SH - m0)
                xtile = xk.tile([P, KC * P], BF)
                nc.sync.dma_start(
                    out=xtile[:].rearrange("p (k m) -> p k m", k=KC)[:, :, 0:mn],
                    in_=xt[:, m0:m0 + mn].rearrange("(k p) m -> p k m", p=P))
                psum = ps.tile([P, HID], FP, space="PSUM")
                for k in range(KC):
                    nc.tensor.matmul(
                        psum[:mn, :],
                        lhsT=xtile[:, k * P:k * P + mn],
                        rhs=w1t[:, k * HID:(k + 1) * HID],
                        start=(k == 0), stop=(k == KC - 1))
                hrow = hp.tile([P, HID], FP)
                hrow_b = hp.tile([P, HID], BF)
                asdt = hp.tile([P, 2], FP)
                if mn < P:
                    nc.vector.memset(hrow[:], 0.0)
                nc.scalar.copy(hrow[:mn, :], psum[:mn, :])
                nc.scalar.copy(hrow_b[:], hrow[:])
                scr1 = hp.tile([P, HID], FP, tag="scratch")
                nc.vector.tensor_tensor(out=scr1[:], in0=hrow[:],
                                        in1=attt[:, 0:HID],
                                        op=mybir.AluOpType.mult)
                nc.vector.tensor_reduce(asdt[:, 0:1], scr1[:],
                                        axis=mybir.AxisListType.X,
                                        op=mybir.AluOpType.add)
                scr2 = hp.tile([P, HID], FP, tag="scratch2")
                nc.vector.tensor_tensor(out=scr2[:], in0=hrow[:],
                                        in1=attt[:, HID:2 * HID],
                                        op=mybir.AluOpType.mult)
                nc.vector.tensor_reduce(asdt[:, 1:2], scr2[:],
                                        axis=mybir.AxisListType.X,
                                        op=mybir.AluOpType.add)
                nc.sync.dma_start(out=hb[m0:m0 + P, :], in_=hrow_b[:])
                nc.sync.dma_start(out=asd[m0:m0 + P, :], in_=asdt[:])

            if repeat > 1:
                with tc.For_i(0, repeat):
                    for t in range(T_TILES):
                        tbody(t)
            else:
                for t in range(T_TILES):
                    tbody(t)
    nc.compile()
    return nc


def _build_aggregate(K, layer, repeat=1, bench_mode=0):
    """Launches 2 & 3: quad gather + segment softmax + weighted aggregation.

    layer == 1:
      out per tile: h' = relu(num + b1); hb2 [SHP, HID] bf16; asd2 [SHP, 2].
      Inputs: tab [NROW, ROWW] bf16, idx [128, sumK*8] i16,
              mask [128, sumK*4] f32, cnts [1, T] i32, adg [SHP, 1] f32,
              vecs [128, 4*HID] f32 = (b1 | u2 | v2 | unused) tiled
    layer == 2:
      out per tile: log_softmax(num @ W2 + b2) -> y [SHP, C_OUT]
      Inputs: tab, idx, mask, cnts, adg, vecs [128, 2*C_OUT] = (b2 | unused),
              w2 [HID, C_OUT]
    repeat > 1 wraps the whole tile loop in a hardware loop (benchmarking).
    """
    nc = bacc.Bacc("TRN2", target_bir_lowering=False, debug=False,
                   num_devices=NCORES, num_swdge_queues=4)
    sumK = int(K.sum())
    tab = nc.dram_tensor("tab", [NROW, ROWW], BF, kind="ExternalInput").ap()
    idx = nc.dram_tensor("idx", [P, sumK * 8], I16, kind="ExternalInput").ap()
    msk = nc.dram_tensor("msk", [P, sumK * 4], FP, kind="ExternalInput").ap()
    cnts = nc.dram_tensor("cnts", [1, T_TILES], I32, kind="ExternalInput").ap()
    adg = nc.dram_tensor("adg", [SHP, 1], FP, kind="ExternalInput").ap()
    if layer == 1:
        vecs = nc.dram_tensor("vecs", [P, 4 * HID], FP, kind="ExternalInput").ap()
        hb2 = nc.dram_tensor("hb2", [SHP, HID], BF, kind="ExternalOutput").ap()
        asd2 = nc.dram_tensor("asd2", [SHP, 2], FP, kind="ExternalOutput").ap()
    else:
        vecs = nc.dram_tensor("vecs", [P, 2 * C_OUT], FP, kind="ExternalInput").ap()
        w2 = nc.dram_tensor("w2", [HID, C_OUT], FP, kind="ExternalInput").ap()
        y = nc.dram_tensor("y", [SHP, C_OUT], FP, kind="ExternalOutput").ap()

    with TileContext(nc) as tc:
        with tc.tile_pool(name="cst", bufs=1) as cst, \
             tc.tile_pool(name="ix", bufs=5) as ixp, \
             tc.tile_pool(name="gr", bufs=5) as grp, \
             tc.tile_pool(name="sc", bufs=3) as scp, \
             tc.tile_pool(name="ou", bufs=3) as oup, \
             tc.tile_pool(name="ps", bufs=2, space="PSUM") as ps:
            vt = cst.tile([P, vecs.shape[1]], FP)
            nc.sync.dma_start(out=vt[:], in_=vecs[:])
            cntt = cst.tile([1, T_TILES], I32)
            nc.sync.dma_start(out=cntt[:], in_=cnts[:])
            if layer == 2:
                w2t = cst.tile([HID, C_OUT], FP)
                nc.sync.dma_start(out=w2t[:], in_=w2[:])
                ident = cst.tile([P, P], FP)
                make_identity(nc, ident[:])
            # zero-fill gather buffers once (stale SBUF could be NaN bits)
            ktmax = int(K.max())
            for b in range(5):
                gz = grp.tile([P, ktmax * ROWW], BF, tag="grid")
                nc.vector.memset(gz[:], 0.0)
            nregs = [nc.gpsimd.alloc_register(f"nidx{i}") for i in range(4)]

            def body(it=None):
                for t in range(T_TILES):
                    kt = int(K[t])
                    o0 = int(K[:t].sum())
                    g = grp.tile([P, kt * ROWW], BF, tag="grid")
                    idx_t = ixp.tile([P, kt * 8], I16, tag="idx")
                    nc.sync.dma_start(out=idx_t[:],
                                      in_=idx[:, o0 * 8:(o0 + kt) * 8])
                    msk_t = ixp.tile([P, kt * 4], FP, tag="msk")
                    nc.sync.dma_start(out=msk_t[:],
                                      in_=msk[:, o0 * 4:(o0 + kt) * 4])
                    adcol = scp.tile([P, 1], FP, tag="adc")
                    nc.sync.dma_start(out=adcol[:], in_=adg[t * P:(t + 1) * P, :])
                    nreg = nregs[t % 4]
                    nc.gpsimd.reg_load(nreg, cntt[0:1, t:t + 1])
                    _my_dma_gather(
                        nc.gpsimd,
                        g[:].rearrange("p (k w) -> p k w", w=ROWW),
                        tab[:, :],
                        idx_t[:],
                        kt * P, nreg, ROWW, ROWW, t % 4)
                    if bench_mode == 1:      # gather-only
                        continue
                    # e = leaky_relu(a_s + a_d) + mask over [P, kt*4]
                    g32 = g[:].bitcast(FP)  # [P, kt*64]
                    as_view = g32.rearrange("p (k s w) -> p k s w",
                                            s=4, w=SUBW // 2)[:, :, :, 8:9]
                    pre = scp.tile([P, kt * 4], FP, tag="pre")
                    e = scp.tile([P, kt * 4], FP, tag="e")
                    m = scp.tile([P, 1], FP, tag="m")
                    nc.vector.tensor_scalar_add(
                        pre[:], as_view.rearrange("p k s w -> p (k s w)"),
                        adcol[:])
                    if bench_mode == 3:
                        continue
                    # leaky_relu(x) = max(0.2*x, x)
                    lrl = scp.tile([P, kt * 4], FP, tag="lrl")
                    nc.vector.scalar_tensor_tensor(
                        out=lrl[:], in0=pre[:], scalar=NEG_SLOPE, in1=pre[:],
                        op0=mybir.AluOpType.mult, op1=mybir.AluOpType.max)
                    if bench_mode == 4:
                        continue
                    # e = pre + mask;  m = -max(e)
                    nc.vector.tensor_tensor(out=e[:], in0=lrl[:], in1=msk_t[:],
                                            op=mybir.AluOpType.add)
                    if bench_mode == 5:
                        continue
                    nc.vector.tensor_reduce(m[:], e[:], axis=mybir.AxisListType.X,
                                            op=mybir.AluOpType.max, negate=True)
                    wts = scp.tile([P, kt * 4], FP, tag="w")
                    den = scp.tile([P, 1], FP, tag="den")
                    nc.scalar.activation(
                        wts[:], e[:], mybir.ActivationFunctionType.Exp,
                        bias=m[:], scale=1.0, accum_out=den[:])
                    inv = scp.tile([P, 1], FP, tag="inv")
                    nc.vector.reciprocal(inv[:], den[:])
                    if bench_mode == 2:      # gather + softmax, no aggregate
                        continue
                    # prod = h * w (unnormalized), h = bf16 sub-rows
                    h_view = g[:].rearrange("p (k s w) -> p k s w",
                                            s=4, w=SUBW)[:, :, :, 0:HID]
                    prod = oup.tile([P, kt * 4 * HID], BF, tag="prod")
                    nc.vector.tensor_tensor(
                        out=prod[:].rearrange("p (k s w) -> p k s w",
                                              s=4, w=HID),
                        in0=h_view,
                        in1=wts[:].rearrange("p (k s) -> p k s", s=4)
                            .to_broadcast([P, kt, 4, HID]),
                        op=mybir.AluOpType.mult)
                    num = oup.tile([P, HID], FP, tag="num")
                    pv = prod[:].rearrange("p (k w) -> p w k", w=HID)
                    nc.vector.tensor_reduce(num[:], pv, axis=mybir.AxisListType.X,
                                            op=mybir.AluOpType.add)
                    if layer == 1:
                        hrow = oup.tile([P, HID], FP, tag="hrow")
                        hrow_b = oup.tile([P, HID], BF, tag="hrowb")
                        asdt = oup.tile([P, 2], FP, tag="asdt")
                        # h' = relu(num/den + b1)
                        nc.vector.scalar_tensor_tensor(
                            out=hrow[:], in0=num[:], scalar=inv[:],
                            in1=vt[:, 0:HID], op0=mybir.AluOpType.mult,
                            op1=mybir.AluOpType.add)
                        nc.vector.tensor_scalar_max(hrow[:], hrow[:], 0.0)
                        nc.scalar.copy(hrow_b[:], hrow[:])
                        scr1 = oup.tile([P, HID], FP, tag="s1")
                        nc.vector.tensor_tensor(out=scr1[:], in0=hrow[:],
                                                in1=vt[:, HID:2 * HID],
                                                op=mybir.AluOpType.mult)
                        nc.vector.tensor_reduce(asdt[:, 0:1], scr1[:],
                                                axis=mybir.AxisListType.X,
                                                op=mybir.AluOpType.add)
                        scr2 = oup.tile([P, HID], FP, tag="s2")
                        nc.vector.tensor_tensor(out=scr2[:], in0=hrow[:],
                                                in1=vt[:, 2 * HID:3 * HID],
                                                op=mybir.AluOpType.mult)
                        nc.vector.tensor_reduce(asdt[:, 1:2], scr2[:],
                                                axis=mybir.AxisListType.X,
                                                op=mybir.AluOpType.add)
                        nc.sync.dma_start(out=hb2[t * P:(t + 1) * P, :],
                                          in_=hrow_b[:])
                        nc.sync.dma_start(out=asd2[t * P:(t + 1) * P, :],
                                          in_=asdt[:])
                    else:
                        pT = ps.tile([HID, P], FP, space="PSUM", tag="pT")
                        nc.tensor.transpose(pT[:], num[:], ident[:])
                        nT = oup.tile([HID, P], FP, tag="nT")
                        nc.scalar.copy(nT[:], pT[:])
                        p2 = ps.tile([P, C_OUT], FP, space="PSUM", tag="p2")
                        nc.tensor.matmul(p2[:], lhsT=nT[:], rhs=w2t[:],
                                         start=True, stop=True)
                        o = oup.tile([P, C_OUT], FP, tag="o")
                        # o = (num @ W2)/den + b2
                        nc.vector.scalar_tensor_tensor(
                            out=o[:], in0=p2[:], scalar=inv[:],
                            in1=vt[:, 0:C_OUT], op0=mybir.AluOpType.mult,
                            op1=mybir.AluOpType.add)
                        mx = scp.tile([P, 1], FP, tag="mx")
                        nc.vector.tensor_reduce(mx[:], o[:],
                                                axis=mybir.AxisListType.X,
                                                op=mybir.AluOpType.max,
                                                negate=True)
                        ex = oup.tile([P, C_OUT], FP, tag="ex")
                        se = scp.tile([P, 1], FP, tag="se")
                        nc.scalar.activation(ex[:], o[:],
                                             mybir.ActivationFunctionType.Exp,
                                             bias=mx[:], scale=1.0,
                                             accum_out=se[:])
                        ls = scp.tile([P, 1], FP, tag="ls")
                        nc.scalar.activation(ls[:], se[:],
                                             mybir.ActivationFunctionType.Ln)
                        ofs = scp.tile([P, 1], FP, tag="ofs")
                        nc.vector.tensor_tensor(out=ofs[:], in0=mx[:], in1=ls[:],
                                                op=mybir.AluOpType.subtract)
                        nc.scalar.activation(o[:], o[:],
                                             mybir.ActivationFunctionType.Identity,
                                             bias=ofs[:], scale=1.0)
                        nc.sync.dma_start(out=y[t * P:(t + 1) * P, :], in_=o[:])

            if repeat > 1:
                with tc.For_i(0, repeat) as it:
                    body(it)
            else:
                body()
    nc.compile()
    return nc


# ---------------------------------------------------------------------------
# Main entry
# ---------------------------------------------------------------------------

LAST_TIMINGS = {}
LAST_STATS = {}


def _run_retry(nc, in_maps, cores):
    try:
        return run_bass_kernel_spmd(nc, in_maps, cores)
    except Exception:
        # transient accelerator-unrecoverable states heal on retry
        return run_bass_kernel_spmd(nc, in_maps, cores)


def _pack_table(h_bf16_bits, a_s):
    """h_bf16_bits [N, HID] uint16, a_s [N] float32 -> [NROW, ROWW] bf16."""
    import ml_dtypes
    tab = np.zeros((NROW, 4, SUBW), np.uint16)
    tab[:, :, 0:HID] = h_bf16_bits.reshape(NROW, 4, HID)
    tab[:, :, HID:HID + 2] = a_s.astype(np.float32).view(np.uint16).reshape(
        NROW, 4, 2)
    return tab.reshape(NROW, ROWW).view(ml_dtypes.bfloat16)


def kernel(x, edge_index, W1, att_src1, att_dst1, b1, W2, att_src2, att_dst2, b2):
    import time as _time
    x = np.asarray(x, np.float32)
    W1 = np.asarray(W1, np.float32)
    W2 = np.asarray(W2, np.float32)
    att_src1 = np.asarray(att_src1, np.float32)
    att_dst1 = np.asarray(att_dst1, np.float32)
    att_src2 = np.asarray(att_src2, np.float32)
    att_dst2 = np.asarray(att_dst2, np.float32)
    b1 = np.asarray(b1, np.float32)
    b2 = np.asarray(b2, np.float32)

    import jax.numpy as jnp

    def to_bf16(a):
        return np.asarray(jnp.asarray(a, dtype=jnp.bfloat16))

    print("preprocess...", flush=True)
    _t = _time.time()
    K, cores_data = _build_schedule(edge_index)
    LAST_STATS["descs_per_core"] = [int(cd["counts"].sum())
                                    for cd in cores_data]
    LAST_STATS["sumK"] = int(K.sum())
    LAST_TIMINGS["preprocess"] = _time.time() - _t

    # ---- launch 1: transform -------------------------------------------
    print("build1...", flush=True)
    nc1 = _build_transform()
    xT_bf = to_bf16(np.ascontiguousarray(x.T))
    att_t = np.tile(np.concatenate([att_src1, att_dst1])[None, :], (P, 1))
    w1r = np.ascontiguousarray(to_bf16(W1).reshape(F_IN // P, P, HID))
    in1 = [{"xt": np.ascontiguousarray(xT_bf[:, c * SH:(c + 1) * SH]),
            "w1": w1r, "att": att_t.astype(np.float32)}
           for c in range(NCORES)]
    _t = _time.time()
    r1 = _run_retry(nc1, in1, list(range(NCORES)))
    LAST_TIMINGS["launch1"] = _time.time() - _t
    print("launch1 done", flush=True)
    h_bits = np.concatenate(
        [np.asarray(r1.results[c]["hb"][:SH]).view(np.uint16)
         for c in range(NCORES)], axis=0)     # [N, HID] u16, node-id order
    asd1 = np.concatenate(
        [r1.results[c]["asd"][:SH] for c in range(NCORES)], axis=0)  # [N, 2]
    tab1 = _pack_table(h_bits, asd1[:, 0])

    # ---- launch 2: layer-1 aggregation ---------------------------------
    print("build2...", flush=True)
    nc2 = _build_aggregate(K, layer=1)
    u2 = W2 @ att_src2
    v2 = W2 @ att_dst2
    vecs1 = np.zeros((P, 4 * HID), np.float32)
    vecs1[:, 0:HID] = b1[None, :]
    vecs1[:, HID:2 * HID] = u2[None, :]
    vecs1[:, 2 * HID:3 * HID] = v2[None, :]
    in2 = []
    for c in range(NCORES):
        cd = cores_data[c]
        adgc = np.zeros((SHP, 1), np.float32)
        adgc[:SH, 0] = asd1[c * SH + cd["order"], 1]
        in2.append({"tab": tab1, "idx": cd["idx"], "msk": cd["mask"],
                    "cnts": cd["counts"], "adg": adgc, "vecs": vecs1})
    _t = _time.time()
    r2 = _run_retry(nc2, in2, list(range(NCORES)))
    LAST_TIMINGS["launch2"] = _time.time() - _t
    print("launch2 done", flush=True)
    h2_bits = np.empty((N, HID), np.uint16)
    asd2 = np.empty((N, 2), np.float32)
    for c in range(NCORES):
        cd = cores_data[c]
        ids = c * SH + cd["order"]
        h2_bits[ids] = np.asarray(r2.results[c]["hb2"][:SH]).view(np.uint16)
        asd2[ids] = r2.results[c]["asd2"][:SH]
    tab2 = _pack_table(h2_bits, asd2[:, 0])

    # ---- launch 3: layer-2 aggregation + classifier --------------------
    print("build3...", flush=True)
    nc3 = _build_aggregate(K, layer=2)
    vecs2 = np.zeros((P, 2 * C_OUT), np.float32)
    vecs2[:, 0:C_OUT] = b2[None, :]
    in3 = []
    for c in range(NCORES):
        cd = cores_data[c]
        adgc = np.zeros((SHP, 1), np.float32)
        adgc[:SH, 0] = asd2[c * SH + cd["order"], 1]
        in3.append({"tab": tab2, "idx": cd["idx"], "msk": cd["mask"],
                    "cnts": cd["counts"], "adg": adgc, "vecs": vecs2,
                    "w2": W2})
    _t = _time.time()
    r3 = _run_retry(nc3, in3, list(range(NCORES)))
    LAST_TIMINGS["launch3"] = _time.time() - _t
    print("launch3 done", flush=True)

    out = np.zeros((N, C_OUT), np.float32)
    for c in range(NCORES):
        cd = cores_data[c]
        out[c * SH + cd["order"]] = r3.results[c]["y"][:SH, :]
    return out
